# revision 14
# baseline (speedup 1.0000x reference)
"""ALiBi multi-head attention on 8 TRN2 NeuronCores.

Problem: x [2, 2048, 1024] fp32, W_kqv [3072, 1024] fp32 (row chunks k,q,v),
16 heads x 64 dim, causal + ALiBi, softmax scale = sqrt(1024) = 32.

Sharding: batch x head-block. Core c handles batch b = c//4 and heads
[4*(c%4), 4*(c%4)+4). Attention is embarrassingly parallel over (b, h):
no collectives; host shards inputs / gathers outputs.

Device-side layout choices (per core):
- Host supplies x[b].T ("xt" [1024, 2048]) and column shards of W_kqv
  pre-transposed, so all matmuls contract over the partition dim with no
  on-device transposes of x/W.
- Q^T/K^T are produced in [d, s] layout (2 heads packed per 128-partition
  tile); scores are computed transposed, S^T[j, i] tiles, so softmax(j)
  runs along the partition dim: no max-subtraction is needed (causal+ALiBi
  bound scores above by ~2), the denominator comes from a ones column
  appended to V (one extra PSUM row in the same matmul), and no transposes
  of the 2048x2048 probability matrix are ever done.
- All matmuls use bf16 operands with fp32 PSUM accumulation (fastest PE
  path that keeps the HAM clock-gate warm; rel err a few e-3).
- ALiBi bias + causal mask come from one precomputed base tile
  PM[p, u] = (p - (u-511)) masked to -1e9 where j > i; per (head, tile)
  the bias is PM scaled by the head slope, indexed with a shifted AP.
"""

import math
import os
import sys

import numpy as np

for _p in ("/opt/trn_rl_repo",):
    if _p not in sys.path:
        sys.path.insert(0, _p)

B, S, E = 2, 2048, 1024
H, D = 16, 64
H_LOC = 4          # heads per core
COLS = H_LOC * D   # 256 output columns per core
SCALE = 1.0 / math.sqrt(E)
N_CORES = 8
NEG = -1e9
PM_W = 2560        # base bias tile width: u in [-511, 2048]

_NC_CACHE = [None]


def _build():
    import concourse.bacc as bacc
    import concourse.mybir as mybir
    import concourse.tile as tile
    from concourse.masks import make_identity

    f32 = mybir.dt.float32
    bf16 = mybir.dt.bfloat16
    nc = bacc.Bacc("TRN2", target_bir_lowering=False, debug=False,
                   num_devices=N_CORES)

    xt = nc.dram_tensor("xt", [E, S], f32, kind="ExternalInput")
    wt_qk = nc.dram_tensor("wt_qk", [E, 2 * COLS], f32, kind="ExternalInput")
    wt_v = nc.dram_tensor("wt_v", [E, COLS], f32, kind="ExternalInput")
    slopes = nc.dram_tensor("slopes", [128, H_LOC], f32, kind="ExternalInput")
    out = nc.dram_tensor("out", [S, COLS], f32, kind="ExternalOutput")

    NE = E // 128     # 8 e-tiles
    NS = S // 512     # 4 s-chunks of 512
    NST = S // 128    # 16 s-tiles of 128

    with tile.TileContext(nc) as tc:
        with tc.tile_pool(name="const", bufs=1) as cpool, \
             tc.tile_pool(name="persist", bufs=1) as pp, \
             tc.tile_pool(name="work", bufs=4) as wp, \
             tc.tile_pool(name="bm", bufs=4) as bmp, \
             tc.tile_pool(name="ps_s", bufs=5, space="PSUM") as ps_s:

            # ---- constants ----
            ident = cpool.tile([128, 128], f32, tag="ident")
            make_identity(nc, ident[:])
            slp = cpool.tile([128, H_LOC], f32, tag="slp")
            nc.sync.dma_start(slp[:], slopes[:, :])
            ones4 = cpool.tile([128, H_LOC, 1], f32, tag="ones4")
            nc.vector.memset(ones4[:], 1.0)

            # PM[p, t]: value 511 + p - t where (t - 511 - p) >= 0 else NEG
            # (fp32 iota is exact here: |values| <= 2559 < 2^24)
            pm = cpool.tile([128, PM_W], f32, tag="pm")
            nc.gpsimd.iota(pm[:], pattern=[[-1, PM_W]], base=511,
                           channel_multiplier=1,
                           allow_small_or_imprecise_dtypes=True)
            nc.gpsimd.affine_select(
                out=pm[:], in_=pm[:], compare_op=mybir.AluOpType.is_ge,
                fill=NEG, base=-511, pattern=[[1, PM_W]], channel_multiplier=-1)

            # ALiBi bias tiles for all 4 heads, generated up front while
            # the DVE idles during the input-DMA window.
            bms = []
            for h in range(H_LOC):
                bm = bmp.tile([128, PM_W], f32, tag="bm", name=f"bm{h}")
                nc.vector.tensor_scalar_mul(bm[:], pm[:], slp[:, h:h + 1])
                bms.append(bm)

            # ---- persistent activations ----
            # Per-head Q^T/K^T [128, S] with the unused 64-partition half
            # zeroed: keeps every score matmul at full K=128 contraction
            # (zeros contribute nothing; matmul cost is N cycles either way)
            # so the PE activity monitor sees a fully-busy array.
            qt = [pp.tile([128, S], bf16, tag=f"qt{h}", name=f"qt{h}")
                  for h in range(H_LOC)]
            kt_t = [pp.tile([128, S], bf16, tag=f"kt{h}", name=f"ktt{h}")
                    for h in range(H_LOC)]
            for h in range(H_LOC):
                z0 = 64 if h % 2 == 0 else 0
                nc.vector.memset(qt[h][z0:z0 + 64, :], 0.0)
                nc.vector.memset(kt_t[h][z0:z0 + 64, :], 0.0)
            va = [pp.tile([128, H_LOC * 65], bf16, tag=f"va{st}", name=f"va{st}")
                  for st in range(NST)]
            os_t = [pp.tile([128, COLS], f32, tag=f"os{st}", name=f"ost{st}")
                    for st in range(NST)]

            # ---- phase 1: QKV projection (inputs scoped to free SBUF) ----
            # Load fp32 via fast HWDGE DMA, cast to bf16 with DVE 4x-mode
            # copies (a casting SWDGE DMA shatters into per-element
            # descriptors, and GpSimd casts measure ~5us per tile).
            with tc.tile_pool(name="inp", bufs=1) as ip, \
                 tc.tile_pool(name="stage", bufs=3) as sp, \
                 tc.tile_pool(name="ps_v", bufs=1, space="PSUM") as ps_v:
                xtr = [ip.tile([128, S], bf16, tag=f"xt{e}", name=f"xtr{e}") for e in range(NE)]
                wqk = [ip.tile([128, 2 * COLS], bf16, tag=f"wqk{e}", name=f"wqk{e}")
                       for e in range(NE)]
                wv = [ip.tile([128, COLS], bf16, tag=f"wv{e}", name=f"wv{e}")
                      for e in range(NE)]
                for e in range(NE):
                    xs = sp.tile([128, S], f32, tag="xs", name="xs")
                    nc.sync.dma_start(xs[:], xt[e * 128:(e + 1) * 128, :])
                    nc.vector.tensor_copy(xtr[e][:], xs[:])
                    ws = sp.tile([128, 2 * COLS], f32, tag="ws", name="ws")
                    nc.sync.dma_start(ws[:], wt_qk[e * 128:(e + 1) * 128, :])
                    nc.vector.tensor_copy(wqk[e][:], ws[:])
                    vs = sp.tile([128, COLS], f32, tag="vs", name="vs")
                    nc.sync.dma_start(vs[:], wt_v[e * 128:(e + 1) * 128, :])
                    nc.vector.tensor_copy(wv[e][:], vs[:])

                # Q^T / K^T: [f, s] layout. f-tiles 0,1 = Q heads (01)(23);
                # 2,3 = K heads. The 1/32 score scale is folded into the Q
                # weights host-side. Emit in f order 0,2,1,3 so heads 0/1
                # unblock the attention phase early. Each psum half-row block
                # goes to its head's padded tile (same partitions - engines
                # cannot move data across partitions). The casts run on the
                # Scalar engine, idle until the first exp.
                def qk_tiles(f):
                    for sc in range(NS):
                        p = ps_s.tile([128, 512], f32, tag="s")
                        for e in range(NE):
                            nc.tensor.matmul(
                                p[:],
                                wqk[e][:, f * 128:(f + 1) * 128],
                                xtr[e][:, sc * 512:(sc + 1) * 512],
                                start=(e == 0), stop=(e == NE - 1))
                        sl = slice(sc * 512, (sc + 1) * 512)
                        dst = qt if f < 2 else kt_t
                        fb = f if f < 2 else f - 2
                        nc.scalar.copy(dst[2 * fb][0:64, sl], p[0:64, :])
                        nc.scalar.copy(dst[2 * fb + 1][64:128, sl], p[64:128, :])

                qk_tiles(0)
                qk_tiles(2)

                # V in [s, d] layout, augmented with a ones column per head.
                for st in range(NST):
                    p = ps_v.tile([128, COLS], f32, tag="v")
                    for e in range(NE):
                        nc.tensor.matmul(
                            p[:],
                            xtr[e][:, st * 128:(st + 1) * 128],
                            wv[e][:],
                            start=(e == 0), stop=(e == NE - 1))
                    var = va[st][:].rearrange("p (h c) -> p h c", h=H_LOC)
                    nc.vector.tensor_copy(
                        var[:, :, 0:64],
                        p[:].rearrange("p (h c) -> p h c", h=H_LOC))
                    nc.vector.tensor_copy(var[:, :, 64:65], ones4[:])

                qk_tiles(1)
                qk_tiles(3)

            # ---- phase 2: attention, two heads interleaved ----
            # qc-outer / kt-inner per head pair: interleaving a head pair
            # keeps an independent score matmul ready whenever the other
            # head waits on its softmax chain.
            def attn_tile(h, bm, qc, kt, po, ktmax):
                ps = ps_s.tile([128, 512], f32, tag="s", name="ps")
                nc.tensor.matmul(
                    ps[:],
                    kt_t[h][:, kt * 128:(kt + 1) * 128],
                    qt[h][:, qc * 512:(qc + 1) * 512],
                    start=True, stop=True)
                u0 = qc * 512 - kt * 128 + 511
                sbb = wp.tile([128, 512], bf16, tag="sbb", name="sbb")
                nc.vector.tensor_add(sbb[:], ps[:], bm[:, u0:u0 + 512])
                et = wp.tile([128, 512], bf16, tag="et", name="et")
                nc.scalar.activation(et[:], sbb[:],
                                     mybir.ActivationFunctionType.Exp)
                nc.tensor.matmul(
                    po[:], va[kt][:, h * 65:(h + 1) * 65], et[:],
                    start=(kt == 0), stop=(kt == ktmax))

            def attn_epilogue(h, qc, po):
                osb = wp.tile([65, 512], f32, tag="osb", name="osb")
                nc.vector.tensor_copy(osb[:], po[:])
                for i in range(4):
                    pt = ps_s.tile([128, 65], f32, tag="s", name="pt",
                                   padded_shape=[128, 512])
                    nc.tensor.transpose(pt[:], osb[:, i * 128:(i + 1) * 128],
                                        ident[0:65, 0:65])
                    rec = wp.tile([128, 1], f32, tag="rec", name="rec")
                    nc.vector.reciprocal(rec[:], pt[:, 64:65])
                    st = qc * 4 + i
                    nc.vector.tensor_scalar_mul(
                        os_t[st][:, h * 64:(h + 1) * 64], pt[:, 0:64],
                        rec[:])

            with tc.tile_pool(name="ps_o", bufs=2, space="PSUM") as ps_o:
                for hp in range(H_LOC // 2):
                    h0, h1 = 2 * hp, 2 * hp + 1
                    bm0, bm1 = bms[h0], bms[h1]
                    for qc in range(NS):
                        ktmax = (qc * 512 + 511) // 128
                        po0 = ps_o.tile([65, 512], f32, tag="o", name="po0")
                        po1 = ps_o.tile([65, 512], f32, tag="o", name="po1")
                        for kt in range(ktmax + 1):
                            attn_tile(h0, bm0, qc, kt, po0, ktmax)
                            attn_tile(h1, bm1, qc, kt, po1, ktmax)
                        attn_epilogue(h0, qc, po0)
                        attn_epilogue(h1, qc, po1)

            # ---- phase 3: store ----
            for st in range(NST):
                nc.sync.dma_start(out[st * 128:(st + 1) * 128, :], os_t[st][:])

    nc.compile()
    return nc


def _get_nc():
    if _NC_CACHE[0] is None:
        _NC_CACHE[0] = _build()
    return _NC_CACHE[0]


def _alibi_slopes():
    x = (2 ** 8) ** (1.0 / H)
    return np.array([1.0 / x ** (i + 1) for i in range(H)], dtype=np.float32)


def kernel(x: np.ndarray, W_kqv: np.ndarray) -> np.ndarray:
    from concourse.bass_utils import run_bass_kernel_spmd

    x = np.asarray(x, dtype=np.float32)
    W_kqv = np.asarray(W_kqv, dtype=np.float32)
    slopes = _alibi_slopes()

    nc = _get_nc()
    in_maps = []
    for c in range(N_CORES):
        b, hb = c // H_LOC, c % H_LOC
        r0 = hb * COLS
        wk = W_kqv[r0:r0 + COLS, :]                 # k rows
        wq = W_kqv[E + r0:E + r0 + COLS, :] * np.float32(SCALE)  # q rows, pre-scaled
        wv = W_kqv[2 * E + r0:2 * E + r0 + COLS, :]  # v rows
        in_maps.append({
            "xt": np.ascontiguousarray(x[b].T),
            "wt_qk": np.ascontiguousarray(
                np.concatenate([wq, wk], axis=0).T),
            "wt_v": np.ascontiguousarray(wv.T),
            "slopes": np.tile(slopes[hb * H_LOC:(hb + 1) * H_LOC], (128, 1)),
        })

    res = run_bass_kernel_spmd(
        nc, in_maps, core_ids=list(range(N_CORES)),
        trace=os.environ.get("BASS_TRACE") == "1")

    outp = np.empty((B, S, E), dtype=np.float32)
    for c in range(N_CORES):
        b, hb = c // H_LOC, c % H_LOC
        outp[b, :, hb * COLS:(hb + 1) * COLS] = res.results[c]["out"]
    if os.environ.get("BASS_TRACE") == "1":
        kernel.last_exec_time_ns = res.exec_time_ns
        kernel.last_results = res
    return outp


# revision 15
# speedup vs baseline: 1.1771x; 1.1771x over previous
"""ALiBi multi-head attention on 8 TRN2 NeuronCores.

Problem: x [2, 2048, 1024] fp32, W_kqv [3072, 1024] fp32 (row chunks k,q,v),
16 heads x 64 dim, causal + ALiBi, softmax scale = sqrt(1024) = 32.

Sharding: batch x head-block. Core c handles batch b = c//4 and heads
[4*(c%4), 4*(c%4)+4). Attention is embarrassingly parallel over (b, h):
no collectives; host shards inputs / gathers outputs.

Device-side layout choices (per core):
- Host supplies x[b].T ("xt" [1024, 2048]) and column shards of W_kqv
  pre-transposed, so all matmuls contract over the partition dim with no
  on-device transposes of x/W.
- Q^T/K^T are produced in [d, s] layout (2 heads packed per 128-partition
  tile); scores are computed transposed, S^T[j, i] tiles, so softmax(j)
  runs along the partition dim: no max-subtraction is needed (causal+ALiBi
  bound scores above by ~2), the denominator comes from a ones column
  appended to V (one extra PSUM row in the same matmul), and no transposes
  of the 2048x2048 probability matrix are ever done.
- All matmuls use bf16 operands with fp32 PSUM accumulation (fastest PE
  path that keeps the HAM clock-gate warm; rel err a few e-3).
- ALiBi bias + causal mask come from one precomputed base tile
  PM[p, u] = (p - (u-511)) masked to -1e9 where j > i; per (head, tile)
  the bias is PM scaled by the head slope, indexed with a shifted AP.
"""

import math
import os
import sys

import numpy as np

for _p in ("/opt/trn_rl_repo",):
    if _p not in sys.path:
        sys.path.insert(0, _p)

B, S, E = 2, 2048, 1024
H, D = 16, 64
H_LOC = 4          # heads per core
COLS = H_LOC * D   # 256 output columns per core
SCALE = 1.0 / math.sqrt(E)
N_CORES = 8
NEG = -1e9
PM_W = 2560        # base bias tile width: u in [-511, 2048]

_NC_CACHE = [None]


def _build():
    import concourse.bacc as bacc
    import concourse.mybir as mybir
    import concourse.tile as tile
    from concourse.masks import make_identity

    f32 = mybir.dt.float32
    bf16 = mybir.dt.bfloat16
    nc = bacc.Bacc("TRN2", target_bir_lowering=False, debug=False,
                   num_devices=N_CORES)

    xt = nc.dram_tensor("xt", [E, S], f32, kind="ExternalInput")
    wt_qk = nc.dram_tensor("wt_qk", [E, 2 * COLS], f32, kind="ExternalInput")
    wt_v = nc.dram_tensor("wt_v", [E, COLS], f32, kind="ExternalInput")
    slopes = nc.dram_tensor("slopes", [128, H_LOC], f32, kind="ExternalInput")
    out = nc.dram_tensor("out", [S, COLS], f32, kind="ExternalOutput")

    NE = E // 128     # 8 e-tiles
    NS = S // 512     # 4 s-chunks of 512
    NST = S // 128    # 16 s-tiles of 128

    with tile.TileContext(nc) as tc:
        with tc.tile_pool(name="const", bufs=1) as cpool, \
             tc.tile_pool(name="persist", bufs=1) as pp, \
             tc.tile_pool(name="work", bufs=4) as wp, \
             tc.tile_pool(name="bm", bufs=2) as bmp, \
             tc.tile_pool(name="ps_s", bufs=5, space="PSUM") as ps_s:

            # ---- constants ----
            ident = cpool.tile([128, 128], f32, tag="ident")
            make_identity(nc, ident[:])
            slp = cpool.tile([128, H_LOC], f32, tag="slp")
            nc.sync.dma_start(slp[:], slopes[:, :])
            ones4 = cpool.tile([128, H_LOC, 1], f32, tag="ones4")
            nc.vector.memset(ones4[:], 1.0)

            # PM[p, t]: value 511 + p - t where (t - 511 - p) >= 0 else NEG
            # (fp32 iota is exact here: |values| <= 2559 < 2^24)
            pm = cpool.tile([128, PM_W], f32, tag="pm")
            nc.gpsimd.iota(pm[:], pattern=[[-1, PM_W]], base=511,
                           channel_multiplier=1,
                           allow_small_or_imprecise_dtypes=True)
            nc.gpsimd.affine_select(
                out=pm[:], in_=pm[:], compare_op=mybir.AluOpType.is_ge,
                fill=NEG, base=-511, pattern=[[1, PM_W]], channel_multiplier=-1)

            # ---- persistent activations ----
            # Per-head Q^T/K^T [128, S] with the unused 64-partition half
            # zeroed: keeps every score matmul at full K=128 contraction
            # (zeros contribute nothing; matmul cost is N cycles either way)
            # so the PE activity monitor sees a fully-busy array.
            qt = [pp.tile([128, S], bf16, tag=f"qt{h}", name=f"qt{h}")
                  for h in range(H_LOC)]
            kt_t = [pp.tile([128, S], bf16, tag=f"kt{h}", name=f"ktt{h}")
                    for h in range(H_LOC)]
            for h in range(H_LOC):
                z0 = 64 if h % 2 == 0 else 0
                nc.vector.memset(qt[h][z0:z0 + 64, :], 0.0)
                nc.vector.memset(kt_t[h][z0:z0 + 64, :], 0.0)
            va = [pp.tile([128, H_LOC * 65], bf16, tag=f"va{st}", name=f"va{st}")
                  for st in range(NST)]
            os_t = [pp.tile([128, COLS], f32, tag=f"os{st}", name=f"ost{st}")
                    for st in range(NST)]

            # ---- phase 1: QKV projection (inputs scoped to free SBUF) ----
            # Load fp32 via fast HWDGE DMA, cast to bf16 with DVE 4x-mode
            # copies (a casting SWDGE DMA shatters into per-element
            # descriptors, and GpSimd casts measure ~5us per tile).
            with tc.tile_pool(name="inp", bufs=1) as ip, \
                 tc.tile_pool(name="stage", bufs=3) as sp, \
                 tc.tile_pool(name="ps_v", bufs=1, space="PSUM") as ps_v:
                xtr = [ip.tile([128, S], bf16, tag=f"xt{e}", name=f"xtr{e}") for e in range(NE)]
                wqk = [ip.tile([128, 2 * COLS], bf16, tag=f"wqk{e}", name=f"wqk{e}")
                       for e in range(NE)]
                wv = [ip.tile([128, COLS], bf16, tag=f"wv{e}", name=f"wv{e}")
                      for e in range(NE)]
                for e in range(NE):
                    xs = sp.tile([128, S], f32, tag="xs", name="xs")
                    nc.sync.dma_start(xs[:], xt[e * 128:(e + 1) * 128, :])
                    nc.vector.tensor_copy(xtr[e][:], xs[:])
                    ws = sp.tile([128, 2 * COLS], f32, tag="ws", name="ws")
                    nc.sync.dma_start(ws[:], wt_qk[e * 128:(e + 1) * 128, :])
                    nc.vector.tensor_copy(wqk[e][:], ws[:])
                    vs = sp.tile([128, COLS], f32, tag="vs", name="vs")
                    nc.sync.dma_start(vs[:], wt_v[e * 128:(e + 1) * 128, :])
                    nc.vector.tensor_copy(wv[e][:], vs[:])

                # Q^T / K^T: [f, s] layout. f-tiles 0,1 = Q heads (01)(23);
                # 2,3 = K heads. The 1/32 score scale is folded into the Q
                # weights host-side. Emit in f order 0,2,1,3 so heads 0/1
                # unblock the attention phase early. Each psum half-row block
                # goes to its head's padded tile (same partitions - engines
                # cannot move data across partitions). The casts run on the
                # Scalar engine, idle until the first exp.
                def qk_tiles(f):
                    for sc in range(NS):
                        p = ps_s.tile([128, 512], f32, tag="s")
                        for e in range(NE):
                            nc.tensor.matmul(
                                p[:],
                                wqk[e][:, f * 128:(f + 1) * 128],
                                xtr[e][:, sc * 512:(sc + 1) * 512],
                                start=(e == 0), stop=(e == NE - 1))
                        sl = slice(sc * 512, (sc + 1) * 512)
                        dst = qt if f < 2 else kt_t
                        fb = f if f < 2 else f - 2
                        nc.scalar.copy(dst[2 * fb][0:64, sl], p[0:64, :])
                        nc.scalar.copy(dst[2 * fb + 1][64:128, sl], p[64:128, :])

                qk_tiles(0)
                qk_tiles(2)

                # V in [s, d] layout, augmented with a ones column per head.
                for st in range(NST):
                    p = ps_v.tile([128, COLS], f32, tag="v")
                    for e in range(NE):
                        nc.tensor.matmul(
                            p[:],
                            xtr[e][:, st * 128:(st + 1) * 128],
                            wv[e][:],
                            start=(e == 0), stop=(e == NE - 1))
                    var = va[st][:].rearrange("p (h c) -> p h c", h=H_LOC)
                    nc.vector.tensor_copy(
                        var[:, :, 0:64],
                        p[:].rearrange("p (h c) -> p h c", h=H_LOC))
                    nc.vector.tensor_copy(var[:, :, 64:65], ones4[:])

                qk_tiles(1)
                qk_tiles(3)

            # ---- phase 2: attention, two heads interleaved ----
            # qc-outer / kt-inner per head pair: interleaving a head pair
            # keeps an independent score matmul ready whenever the other
            # head waits on its softmax chain.
            def attn_tile(h, bm, qc, kt, po, ktmax):
                ps = ps_s.tile([128, 512], f32, tag="s", name="ps")
                nc.tensor.matmul(
                    ps[:],
                    kt_t[h][:, kt * 128:(kt + 1) * 128],
                    qt[h][:, qc * 512:(qc + 1) * 512],
                    start=True, stop=True)
                u0 = qc * 512 - kt * 128 + 511
                sbb = wp.tile([128, 512], bf16, tag="sbb", name="sbb")
                nc.vector.tensor_add(sbb[:], ps[:], bm[:, u0:u0 + 512])
                et = wp.tile([128, 512], bf16, tag="et", name="et")
                nc.scalar.activation(et[:], sbb[:],
                                     mybir.ActivationFunctionType.Exp)
                nc.tensor.matmul(
                    po[:], va[kt][:, h * 65:(h + 1) * 65], et[:],
                    start=(kt == 0), stop=(kt == ktmax))

            def attn_epilogue(h, qc, po):
                osb = wp.tile([65, 512], f32, tag="osb", name="osb")
                nc.vector.tensor_copy(osb[:], po[:])
                for i in range(4):
                    pt = ps_s.tile([128, 65], f32, tag="s", name="pt",
                                   padded_shape=[128, 512])
                    nc.tensor.transpose(pt[:], osb[:, i * 128:(i + 1) * 128],
                                        ident[0:65, 0:65])
                    rec = wp.tile([128, 1], f32, tag="rec", name="rec")
                    nc.vector.reciprocal(rec[:], pt[:, 64:65])
                    st = qc * 4 + i
                    nc.vector.tensor_scalar_mul(
                        os_t[st][:, h * 64:(h + 1) * 64], pt[:, 0:64],
                        rec[:])

            with tc.tile_pool(name="ps_o", bufs=2, space="PSUM") as ps_o:
                for hp in range(H_LOC // 2):
                    h0, h1 = 2 * hp, 2 * hp + 1
                    bm0 = bmp.tile([128, PM_W], f32, tag="bm", name="bm0")
                    nc.vector.tensor_scalar_mul(bm0[:], pm[:],
                                                slp[:, h0:h0 + 1])
                    bm1 = bmp.tile([128, PM_W], f32, tag="bm", name="bm1")
                    nc.vector.tensor_scalar_mul(bm1[:], pm[:],
                                                slp[:, h1:h1 + 1])
                    for qc in range(NS):
                        ktmax = (qc * 512 + 511) // 128
                        po0 = ps_o.tile([65, 512], f32, tag="o", name="po0")
                        po1 = ps_o.tile([65, 512], f32, tag="o", name="po1")
                        for kt in range(ktmax + 1):
                            attn_tile(h0, bm0, qc, kt, po0, ktmax)
                            attn_tile(h1, bm1, qc, kt, po1, ktmax)
                        attn_epilogue(h0, qc, po0)
                        attn_epilogue(h1, qc, po1)

            # ---- phase 3: store ----
            for st in range(NST):
                nc.sync.dma_start(out[st * 128:(st + 1) * 128, :], os_t[st][:])

    nc.compile()
    return nc


def _get_nc():
    if _NC_CACHE[0] is None:
        _NC_CACHE[0] = _build()
    return _NC_CACHE[0]


def _alibi_slopes():
    x = (2 ** 8) ** (1.0 / H)
    return np.array([1.0 / x ** (i + 1) for i in range(H)], dtype=np.float32)


def kernel(x: np.ndarray, W_kqv: np.ndarray) -> np.ndarray:
    from concourse.bass_utils import run_bass_kernel_spmd

    x = np.asarray(x, dtype=np.float32)
    W_kqv = np.asarray(W_kqv, dtype=np.float32)
    slopes = _alibi_slopes()

    nc = _get_nc()
    in_maps = []
    for c in range(N_CORES):
        b, hb = c // H_LOC, c % H_LOC
        r0 = hb * COLS
        wk = W_kqv[r0:r0 + COLS, :]                 # k rows
        wq = W_kqv[E + r0:E + r0 + COLS, :] * np.float32(SCALE)  # q rows, pre-scaled
        wv = W_kqv[2 * E + r0:2 * E + r0 + COLS, :]  # v rows
        in_maps.append({
            "xt": np.ascontiguousarray(x[b].T),
            "wt_qk": np.ascontiguousarray(
                np.concatenate([wq, wk], axis=0).T),
            "wt_v": np.ascontiguousarray(wv.T),
            "slopes": np.tile(slopes[hb * H_LOC:(hb + 1) * H_LOC], (128, 1)),
        })

    res = run_bass_kernel_spmd(
        nc, in_maps, core_ids=list(range(N_CORES)),
        trace=os.environ.get("BASS_TRACE") == "1")

    outp = np.empty((B, S, E), dtype=np.float32)
    for c in range(N_CORES):
        b, hb = c // H_LOC, c % H_LOC
        outp[b, :, hb * COLS:(hb + 1) * COLS] = res.results[c]["out"]
    if os.environ.get("BASS_TRACE") == "1":
        kernel.last_exec_time_ns = res.exec_time_ns
        kernel.last_results = res
    return outp
